# revision 13
# baseline (speedup 1.0000x reference)
"""NT-Xent (SimCLR) contrastive loss on 8 Trainium2 NeuronCores.

Reference computation (B=4096, D=256, T=0.5):
    z   = concat(l2norm(x_i), l2norm(x_j))        # [8192, 256]
    sim = z @ z.T                                  # [8192, 8192]
    pos = diag(sim, +B) ++ diag(sim, -B)           # [8192]
    denom_r = sum_{j != r} exp(sim_rj / T)
    loss = mean(-pos/T + log(denom))

Sharding: row-parallel. Core c owns 1024 rows of z. Each core receives
xall rotated by -c*1024 rows so the SPMD program is identical on every
core. Host pre-casts to bf16 (layout prep only; all math on device).

Device program per core (fused strip pipeline, engines decoupled):
  Per 1024-row strip s (8 strips):
    - gpsimd: plain bf16 DMA load [128, 8, 256]
    - DVE: fused square+rowsum (STT accum) -> nsq
    - DVE: Quake rsqrt bit-trick (shift/int ops, x1.0173 centering
      folded into the scale pass) -> u, keeping ACT's queue free
    - DVE+gpsimd: per-tile scale -> zhat bf16 (strips 0,4 kept for pos)
    - gpsimd: store strip to DRAM; sync: 2 xbar transpose-loads -> zT
  After each odd strip, its 2048-column group gc is ready:
    - per row-tile m: 8 bf16 matmuls (N=512, K=2x128) -> PSUM [128,2048]
    - ACT: Exp(scale=2) with fused accum -> per-row partial sums, OR
      (last tiles) DVE Schraudolph bit-trick exp + bitcast accum pass,
      so ACT and DVE drain exp tiles concurrently.
  Tail: denom = rowsum - e^2, ACT Ln, DVE pos-dots, combine -> [128,1].

Host: loss = sum(core partials) / 8192.
"""

import numpy as np

P = 128
D = 256
B = 4096
R = 2 * B                 # 8192 rows of z
NCORES = 8
BLK = R // NCORES         # 1024 rows per core
NS = 8                    # strips of 1024 rows
TPS = 8                   # 128-row tiles per strip
T_INV = 2.0               # 1 / TEMP
E2 = float(np.exp(T_INV)) # exp(sim_rr / T) with sim_rr == 1
CG = 2048                 # column group width (PSUM tile free dim)
NG = R // CG              # 4 column groups
KCH = D // P              # 2 contraction chunks of 128
BLK_TILES = BLK // P      # 8 row tiles per core block

# Quake rsqrt: y0 = bitcast(0x5f3759df - (bits(x) >> 1)); centered by
# multiplying 1.0173 (folded into the scale pass). Max rel err ~1.8%.
RSQRT_MAGIC = 0x5F3759DF
RSQRT_FIX = 1.0173

# Schraudolph exp(y) ~= bitcast(int32(y * 2^23/ln2 + (127*2^23 - 486408)))
SCH_A = float(T_INV * (1 << 23) / np.log(2.0))   # folds the /T scale
SCH_B = float(127 * (1 << 23) - 486408)

# (gc, m) exp tiles computed on DVE instead of ACT (late tiles; DVE is
# free after the normalize pipeline drains). Interleaved with ACT tiles
# so the two engines drain alternating PSUM banks concurrently.
DVE_TILES = {(3, 1), (3, 3), (3, 5), (3, 6)}

_cached = None


def _build():
    import concourse.bacc as bacc
    import concourse.mybir as mybir
    from concourse import tile

    f32 = mybir.dt.float32
    bf16 = mybir.dt.bfloat16
    i32 = mybir.dt.int32
    AF = mybir.ActivationFunctionType
    ALU = mybir.AluOpType

    # Steer every activation to the one table set containing both Exp and
    # Ln so the kernel performs a single ACT_TABLE_LOAD.
    from concourse import hw_specs as _hw

    _orig_gat = _hw.get_activation_tables

    def _gat_patched(arch):
        tabs = _orig_gat(arch)
        for name, fns in tabs.items():
            if name != "natural_log_exp_and_others":
                fns.discard(AF.Exp)
                fns.discard(AF.Ln)
        return tabs

    bacc.get_activation_tables = _gat_patched

    nc = bacc.Bacc(None, target_bir_lowering=False, debug=False)
    xall = nc.dram_tensor("xall", [R, D], bf16, kind="ExternalInput")
    out_d = nc.dram_tensor("out", [P, 1], f32, kind="ExternalOutput")

    def _emit(tc):
        with (
            tc.tile_pool(name="xpool", bufs=4) as xpool,
            tc.tile_pool(name="zpool", bufs=2) as zpool,
            tc.tile_pool(name="zkeep", bufs=2) as zkpool,
            tc.tile_pool(name="small", bufs=1) as small,
            tc.tile_pool(name="scratch", bufs=1) as scratch,
            tc.tile_pool(name="ztp", bufs=1) as ztp,
            tc.tile_pool(name="escp", bufs=2) as escp,
            tc.tile_pool(name="itp", bufs=2) as itp,
            tc.tile_pool(name="dramp", bufs=1, space="DRAM") as dramp,
            tc.tile_pool(name="psum", bufs=2, space="PSUM") as psum,
        ):
            nsq = small.tile([P, NS * TPS], f32, name="nsq")
            ub = small.tile([P, NS * TPS], i32, name="ub")   # rsqrt bits
            acc = small.tile([P, BLK_TILES * NG], f32, name="acc")
            zhat_dram = [
                dramp.tile([BLK, D], bf16, name=f"zhat{s}", tag=f"zh{s}")
                for s in range(NS)
            ]
            zT = [
                ztp.tile([P, R], bf16, name="zT0", tag="zT0"),
                ztp.tile([P, R], bf16, name="zT1", tag="zT1"),
            ]
            posd = small.tile([P, BLK_TILES], f32, name="posd")
            u_ap = ub[:].bitcast(f32)
            zk = {}
            xs = {}

            def emit_load(s):
                xg = xpool.tile([P, TPS, D], bf16, name=f"xs{s}", tag="xs",
                                bufs=8)
                src = xall[s * BLK:(s + 1) * BLK, :]
                nc.gpsimd.dma_start(xg[:],
                                    src.rearrange("(t p) d -> p t d", p=P))
                xs[s] = xg

            def emit_gc(gc):
                for m in range(BLK_TILES):
                    ps = psum.tile([P, CG], f32, name="ps", tag="ps", bufs=2)
                    for k in range(KCH):
                        lhs = zT[k][:, m * P:(m + 1) * P]
                        for s5 in range(CG // 512):
                            c0 = gc * CG + s5 * 512
                            mm = nc.tensor.matmul(
                                ps[:, s5 * 512:(s5 + 1) * 512], lhs,
                                zT[k][:, c0:c0 + 512],
                                start=(k == 0), stop=(k == KCH - 1))
                            if s5 > 0:
                                mm.ins.ldweights = False
                    col = acc[:, m * NG + gc:m * NG + gc + 1]
                    if (gc, m) in DVE_TILES:
                        # Schraudolph exp on DVE: int32(ps*A+B) then a
                        # second pass over the fp32 bitcast with fused
                        # row-sum accumulation.
                        it = itp.tile([P, CG], i32, name="it", tag="it",
                                      bufs=2)
                        nc.vector.tensor_scalar(
                            out=it[:], in0=ps[:], scalar1=SCH_A,
                            scalar2=SCH_B, op0=ALU.mult, op1=ALU.add)
                        dm = itp.tile([P, CG], bf16, name="dm", tag="dm",
                                      bufs=2)
                        nc.vector.tensor_scalar(
                            out=dm[:], in0=it[:].bitcast(f32), scalar1=1.0,
                            scalar2=0.0, op0=ALU.mult, op1=ALU.add,
                            accum_out=col)
                    else:
                        esc = escp.tile([P, CG], bf16, name="esc", tag="esc",
                                        bufs=2)
                        nc.scalar.activation(
                            esc[:], ps[:], AF.Exp, scale=T_INV,
                            accum_out=col)

            # ---------------- fused strip pipeline ----------------
            # All loads issued up front on the gpsimd queue; the full
            # bf16 input (4 MB = 32 KB/partition) stays SBUF-resident.
            for s in range(NS):
                emit_load(s)
            for s in range(NS):
                xg = xs[s]
                # fused square + row-sum per 128-row tile
                for t in range(TPS):
                    j = s * TPS + t
                    sqs = scratch.tile([P, D], bf16, name="sqs", tag="sqs",
                                       bufs=2)
                    nc.vector.scalar_tensor_tensor(
                        out=sqs[:], in0=xg[:, t, :], scalar=1.0,
                        in1=xg[:, t, :], op0=ALU.mult, op1=ALU.mult,
                        accum_out=nsq[:, j:j + 1])
                # Quake rsqrt on DVE (2 int ops on [P, 8])
                sl = slice(s * TPS, (s + 1) * TPS)
                sh = scratch.tile([P, TPS], i32, name="sh", tag="sh", bufs=2)
                nc.vector.tensor_scalar(
                    out=sh[:], in0=nsq[:, sl].bitcast(i32), scalar1=1,
                    scalar2=None, op0=ALU.logical_shift_right)
                nc.vector.tensor_scalar(
                    out=ub[:, sl], in0=sh[:], scalar1=-1,
                    scalar2=RSQRT_MAGIC, op0=ALU.mult, op1=ALU.add)
                # scale pass -> zhat bf16 (DVE/gpsimd split), keep 0 and 4
                keep = s in (0, 4)
                if keep:
                    zg = zkpool.tile([P, TPS, D], bf16, name=f"zk{s}",
                                     tag="zk", bufs=2)
                    zk[s] = zg
                else:
                    zg = zpool.tile([P, TPS, D], bf16, name="zg", tag="zg",
                                    bufs=2)
                for t in range(TPS):
                    j = s * TPS + t
                    eng = nc.vector if t % 2 == 0 else nc.gpsimd
                    eng.tensor_scalar(
                        out=zg[:, t, :], in0=xg[:, t, :],
                        scalar1=u_ap[:, j:j + 1], scalar2=RSQRT_FIX,
                        op0=ALU.mult, op1=ALU.mult)
                dst = zhat_dram[s][:, :]
                nc.sync.dma_start(
                    dst.rearrange("(t p) d -> p t d", p=P), zg[:])
                rs = slice(s * BLK, (s + 1) * BLK)
                for k in range(KCH):
                    nc.sync.dma_start_transpose(
                        zT[k][:, rs], zhat_dram[s][:, k * P:(k + 1) * P])
                if s % 2 == 1:
                    emit_gc(s // 2)

            # ---------------- Phase C ----------------
            dsum = small.tile([P, BLK_TILES], f32, name="dsum")
            nc.vector.tensor_reduce(
                dsum[:], acc[:].rearrange("p (m g) -> p m g", g=NG),
                axis=mybir.AxisListType.X, op=ALU.add)
            dsub = small.tile([P, BLK_TILES], f32, name="dsub")
            nc.vector.tensor_scalar_add(dsub[:], dsum[:], -E2)
            lnd = small.tile([P, BLK_TILES], f32, name="lnd")
            nc.scalar.activation(lnd[:], dsub[:], AF.Ln)

            for t in range(TPS):
                pscr = scratch.tile([P, D], f32, name="pscr", tag="pscr",
                                    bufs=2)
                nc.vector.scalar_tensor_tensor(
                    out=pscr[:], in0=zk[0][:, t, :], scalar=1.0,
                    in1=zk[4][:, t, :], op0=ALU.mult, op1=ALU.mult,
                    accum_out=posd[:, t:t + 1])

            l1 = small.tile([P, 1], f32, name="l1")
            nc.vector.tensor_reduce(l1[:], lnd[:], axis=mybir.AxisListType.X,
                                    op=ALU.add)
            p1 = small.tile([P, 1], f32, name="p1")
            nc.vector.tensor_reduce(p1[:], posd[:], axis=mybir.AxisListType.X,
                                    op=ALU.add)
            p2 = small.tile([P, 1], f32, name="p2")
            nc.vector.tensor_scalar_mul(p2[:], p1[:], -T_INV)
            comb = small.tile([P, 1], f32, name="comb")
            nc.vector.tensor_add(comb[:], l1[:], p2[:])
            nc.sync.dma_start(out_d[:, :], comb[:])

    with tile.TileContext(nc) as tc:
        _emit(tc)
    nc.compile()
    return nc


def _get_nc():
    global _cached
    if _cached is None:
        _cached = _build()
    return _cached


def _to_bf16(a):
    """Round-to-nearest-even f32 -> bf16 bit pattern, kept as ml_dtypes
    bfloat16 if available, else uint16 view trick via jax-free numpy."""
    import ml_dtypes
    return a.astype(ml_dtypes.bfloat16)


def _make_in_maps(x_i, x_j):
    xall = np.concatenate(
        [np.asarray(x_i, dtype=np.float32), np.asarray(x_j, dtype=np.float32)],
        axis=0,
    )
    xall = _to_bf16(xall)
    return [
        {"xall": np.ascontiguousarray(np.roll(xall, -c * BLK, axis=0))}
        for c in range(NCORES)
    ]


def run(x_i, x_j, trace=False, tmpdir=None):
    """Run on the 8 NeuronCores; returns (loss, BassKernelResults)."""
    from concourse import bass_utils

    nc = _get_nc()
    in_maps = _make_in_maps(x_i, x_j)
    res = bass_utils.run_bass_kernel_spmd(
        nc, in_maps, core_ids=list(range(NCORES)), trace=trace, tmpdir=tmpdir,
    )
    total = np.float32(0.0)
    for r in res.results:
        total += np.float32(np.sum(r["out"], dtype=np.float32))
    loss = np.float32(total / np.float32(R))
    return loss, res


def kernel(x_i, x_j):
    loss, _ = run(x_i, x_j, trace=False)
    return loss


# revision 14
# speedup vs baseline: 1.0509x; 1.0509x over previous
"""NT-Xent (SimCLR) contrastive loss on 8 Trainium2 NeuronCores.

Reference computation (B=4096, D=256, T=0.5):
    z   = concat(l2norm(x_i), l2norm(x_j))        # [8192, 256]
    sim = z @ z.T                                  # [8192, 8192]
    pos = diag(sim, +B) ++ diag(sim, -B)           # [8192]
    denom_r = sum_{j != r} exp(sim_rj / T)
    loss = mean(-pos/T + log(denom))

Sharding: row-parallel. Core c owns 1024 rows of z. Each core receives
xall rotated by -c*1024 rows so the SPMD program is identical on every
core. Host pre-casts to bf16 (layout prep only; all math on device).

Device program per core (fused strip pipeline, engines decoupled):
  Per 1024-row strip s (8 strips):
    - gpsimd: plain bf16 DMA load [128, 8, 256]
    - DVE: fused square+rowsum (STT accum) -> nsq
    - DVE: Quake rsqrt bit-trick (shift/int ops, x1.0173 centering
      folded into the scale pass) -> u, keeping ACT's queue free
    - DVE+gpsimd: per-tile scale -> zhat bf16 (strips 0,4 kept for pos)
    - sync: store strip to its own DRAM tile + 2 xbar transpose-loads
      -> zT (per-strip DRAM tiles break false store->transpose WARs)
  After each odd strip, its 2048-column group gc is ready:
    - per row-tile m: 8 bf16 matmuls (N=512, K=2x128) -> PSUM [128,2048]
    - ACT: Exp(scale=2) with fused accum -> per-row partial sums, OR
      (last tiles) DVE Schraudolph bit-trick exp + bitcast accum pass,
      so ACT and DVE drain exp tiles concurrently.
  Tail: denom = rowsum - e^2, ACT Ln, DVE pos-dots, combine -> [128,1].

Host: loss = sum(core partials) / 8192.
"""

import numpy as np

P = 128
D = 256
B = 4096
R = 2 * B                 # 8192 rows of z
NCORES = 8
BLK = R // NCORES         # 1024 rows per core
NS = 8                    # strips of 1024 rows
TPS = 8                   # 128-row tiles per strip
T_INV = 2.0               # 1 / TEMP
E2 = float(np.exp(T_INV)) # exp(sim_rr / T) with sim_rr == 1
CG = 2048                 # column group width (PSUM tile free dim)
NG = R // CG              # 4 column groups
KCH = D // P              # 2 contraction chunks of 128
BLK_TILES = BLK // P      # 8 row tiles per core block

# Quake rsqrt: y0 = bitcast(0x5f3759df - (bits(x) >> 1)); centered by
# multiplying 1.0173 (folded into the scale pass). Max rel err ~1.8%.
RSQRT_MAGIC = 0x5F3759DF
RSQRT_FIX = 1.0173

# Schraudolph exp(y) ~= bitcast(int32(y * 2^23/ln2 + (127*2^23 - 486408)))
SCH_A = float(T_INV * (1 << 23) / np.log(2.0))   # folds the /T scale
SCH_B = float(127 * (1 << 23) - 486408)

# (gc, m) exp tiles computed on DVE instead of ACT (late tiles; DVE is
# free after the normalize pipeline drains). Interleaved with ACT tiles
# so the two engines drain alternating PSUM banks concurrently.
DVE_TILES = {(3, 1), (3, 3), (3, 5), (3, 6)}

_cached = None


def _build():
    import concourse.bacc as bacc
    import concourse.mybir as mybir
    from concourse import tile

    f32 = mybir.dt.float32
    bf16 = mybir.dt.bfloat16
    i32 = mybir.dt.int32
    AF = mybir.ActivationFunctionType
    ALU = mybir.AluOpType

    # Steer every activation to the one table set containing both Exp and
    # Ln so the kernel performs a single ACT_TABLE_LOAD.
    from concourse import hw_specs as _hw

    _orig_gat = _hw.get_activation_tables

    def _gat_patched(arch):
        tabs = _orig_gat(arch)
        for name, fns in tabs.items():
            if name != "natural_log_exp_and_others":
                fns.discard(AF.Exp)
                fns.discard(AF.Ln)
        return tabs

    bacc.get_activation_tables = _gat_patched

    nc = bacc.Bacc(None, target_bir_lowering=False, debug=False)
    xall = nc.dram_tensor("xall", [R, D], bf16, kind="ExternalInput")
    out_d = nc.dram_tensor("out", [P, 1], f32, kind="ExternalOutput")

    def _emit(tc):
        with (
            tc.tile_pool(name="xpool", bufs=4) as xpool,
            tc.tile_pool(name="zpool", bufs=2) as zpool,
            tc.tile_pool(name="zkeep", bufs=2) as zkpool,
            tc.tile_pool(name="small", bufs=1) as small,
            tc.tile_pool(name="scratch", bufs=1) as scratch,
            tc.tile_pool(name="ztp", bufs=1) as ztp,
            tc.tile_pool(name="escp", bufs=2) as escp,
            tc.tile_pool(name="itp", bufs=2) as itp,
            tc.tile_pool(name="dramp", bufs=1, space="DRAM") as dramp,
            tc.tile_pool(name="psum", bufs=2, space="PSUM") as psum,
        ):
            nsq = small.tile([P, NS * TPS], f32, name="nsq")
            ub = small.tile([P, NS * TPS], i32, name="ub")   # rsqrt bits
            acc = small.tile([P, BLK_TILES * NG], f32, name="acc")
            zhat_dram = [
                dramp.tile([BLK, D], bf16, name=f"zhat{s}", tag=f"zh{s}")
                for s in range(NS)
            ]
            zT = [
                ztp.tile([P, R], bf16, name="zT0", tag="zT0"),
                ztp.tile([P, R], bf16, name="zT1", tag="zT1"),
            ]
            posd = small.tile([P, BLK_TILES], f32, name="posd")
            u_ap = ub[:].bitcast(f32)
            zk = {}
            xs = {}

            def emit_load(s):
                xg = xpool.tile([P, TPS, D], bf16, name=f"xs{s}", tag="xs",
                                bufs=8)
                src = xall[s * BLK:(s + 1) * BLK, :]
                nc.gpsimd.dma_start(xg[:],
                                    src.rearrange("(t p) d -> p t d", p=P))
                xs[s] = xg

            def emit_gc(gc):
                for m in range(BLK_TILES):
                    ps = psum.tile([P, CG], f32, name="ps", tag="ps", bufs=2)
                    for k in range(KCH):
                        lhs = zT[k][:, m * P:(m + 1) * P]
                        for s5 in range(CG // 512):
                            c0 = gc * CG + s5 * 512
                            mm = nc.tensor.matmul(
                                ps[:, s5 * 512:(s5 + 1) * 512], lhs,
                                zT[k][:, c0:c0 + 512],
                                start=(k == 0), stop=(k == KCH - 1))
                            if s5 > 0:
                                mm.ins.ldweights = False
                    col = acc[:, m * NG + gc:m * NG + gc + 1]
                    if (gc, m) in DVE_TILES:
                        # Schraudolph exp on DVE: int32(ps*A+B) then a
                        # second pass over the fp32 bitcast with fused
                        # row-sum accumulation.
                        it = itp.tile([P, CG], i32, name="it", tag="it",
                                      bufs=2)
                        nc.vector.tensor_scalar(
                            out=it[:], in0=ps[:], scalar1=SCH_A,
                            scalar2=SCH_B, op0=ALU.mult, op1=ALU.add)
                        dm = itp.tile([P, CG], bf16, name="dm", tag="dm",
                                      bufs=2)
                        nc.vector.tensor_scalar(
                            out=dm[:], in0=it[:].bitcast(f32), scalar1=1.0,
                            scalar2=0.0, op0=ALU.mult, op1=ALU.add,
                            accum_out=col)
                    else:
                        esc = escp.tile([P, CG], bf16, name="esc", tag="esc",
                                        bufs=2)
                        nc.scalar.activation(
                            esc[:], ps[:], AF.Exp, scale=T_INV,
                            accum_out=col)

            # ---------------- fused strip pipeline ----------------
            # All loads issued up front on the gpsimd queue; the full
            # bf16 input (4 MB = 32 KB/partition) stays SBUF-resident.
            for s in range(NS):
                emit_load(s)
            for s in range(NS):
                xg = xs[s]
                # fused square + row-sum per 128-row tile
                for t in range(TPS):
                    j = s * TPS + t
                    sqs = scratch.tile([P, D], bf16, name="sqs", tag="sqs",
                                       bufs=2)
                    nc.vector.scalar_tensor_tensor(
                        out=sqs[:], in0=xg[:, t, :], scalar=1.0,
                        in1=xg[:, t, :], op0=ALU.mult, op1=ALU.mult,
                        accum_out=nsq[:, j:j + 1])
                # Quake rsqrt on DVE (2 int ops on [P, 8])
                sl = slice(s * TPS, (s + 1) * TPS)
                sh = scratch.tile([P, TPS], i32, name="sh", tag="sh", bufs=2)
                nc.vector.tensor_scalar(
                    out=sh[:], in0=nsq[:, sl].bitcast(i32), scalar1=1,
                    scalar2=None, op0=ALU.logical_shift_right)
                nc.vector.tensor_scalar(
                    out=ub[:, sl], in0=sh[:], scalar1=-1,
                    scalar2=RSQRT_MAGIC, op0=ALU.mult, op1=ALU.add)
                # scale pass -> zhat bf16 (DVE/gpsimd split), keep 0 and 4
                keep = s in (0, 4)
                if keep:
                    zg = zkpool.tile([P, TPS, D], bf16, name=f"zk{s}",
                                     tag="zk", bufs=2)
                    zk[s] = zg
                else:
                    zg = zpool.tile([P, TPS, D], bf16, name="zg", tag="zg",
                                    bufs=2)
                for t in range(TPS):
                    j = s * TPS + t
                    eng = nc.vector if t % 2 == 0 else nc.gpsimd
                    eng.tensor_scalar(
                        out=zg[:, t, :], in0=xg[:, t, :],
                        scalar1=u_ap[:, j:j + 1], scalar2=RSQRT_FIX,
                        op0=ALU.mult, op1=ALU.mult)
                dst = zhat_dram[s][:, :]
                nc.sync.dma_start(
                    dst.rearrange("(t p) d -> p t d", p=P), zg[:])
                rs = slice(s * BLK, (s + 1) * BLK)
                for k in range(KCH):
                    nc.sync.dma_start_transpose(
                        zT[k][:, rs], zhat_dram[s][:, k * P:(k + 1) * P])
                if s % 2 == 1:
                    emit_gc(s // 2)

            # ---------------- Phase C ----------------
            dsum = small.tile([P, BLK_TILES], f32, name="dsum")
            nc.vector.tensor_reduce(
                dsum[:], acc[:].rearrange("p (m g) -> p m g", g=NG),
                axis=mybir.AxisListType.X, op=ALU.add)
            dsub = small.tile([P, BLK_TILES], f32, name="dsub")
            nc.vector.tensor_scalar_add(dsub[:], dsum[:], -E2)
            lnd = small.tile([P, BLK_TILES], f32, name="lnd")
            nc.scalar.activation(lnd[:], dsub[:], AF.Ln)

            for t in range(TPS):
                pscr = scratch.tile([P, D], f32, name="pscr", tag="pscr",
                                    bufs=2)
                nc.vector.scalar_tensor_tensor(
                    out=pscr[:], in0=zk[0][:, t, :], scalar=1.0,
                    in1=zk[4][:, t, :], op0=ALU.mult, op1=ALU.mult,
                    accum_out=posd[:, t:t + 1])

            l1 = small.tile([P, 1], f32, name="l1")
            nc.vector.tensor_reduce(l1[:], lnd[:], axis=mybir.AxisListType.X,
                                    op=ALU.add)
            p1 = small.tile([P, 1], f32, name="p1")
            nc.vector.tensor_reduce(p1[:], posd[:], axis=mybir.AxisListType.X,
                                    op=ALU.add)
            p2 = small.tile([P, 1], f32, name="p2")
            nc.vector.tensor_scalar_mul(p2[:], p1[:], -T_INV)
            comb = small.tile([P, 1], f32, name="comb")
            nc.vector.tensor_add(comb[:], l1[:], p2[:])
            nc.sync.dma_start(out_d[:, :], comb[:])

    with tile.TileContext(nc) as tc:
        _emit(tc)
    nc.compile()
    return nc


def _get_nc():
    global _cached
    if _cached is None:
        _cached = _build()
    return _cached


def _to_bf16(a):
    """Round-to-nearest-even f32 -> bf16 bit pattern, kept as ml_dtypes
    bfloat16 if available, else uint16 view trick via jax-free numpy."""
    import ml_dtypes
    return a.astype(ml_dtypes.bfloat16)


def _make_in_maps(x_i, x_j):
    xall = np.concatenate(
        [np.asarray(x_i, dtype=np.float32), np.asarray(x_j, dtype=np.float32)],
        axis=0,
    )
    xall = _to_bf16(xall)
    return [
        {"xall": np.ascontiguousarray(np.roll(xall, -c * BLK, axis=0))}
        for c in range(NCORES)
    ]


def run(x_i, x_j, trace=False, tmpdir=None):
    """Run on the 8 NeuronCores; returns (loss, BassKernelResults)."""
    from concourse import bass_utils

    nc = _get_nc()
    in_maps = _make_in_maps(x_i, x_j)
    res = bass_utils.run_bass_kernel_spmd(
        nc, in_maps, core_ids=list(range(NCORES)), trace=trace, tmpdir=tmpdir,
    )
    total = np.float32(0.0)
    for r in res.results:
        total += np.float32(np.sum(r["out"], dtype=np.float32))
    loss = np.float32(total / np.float32(R))
    return loss, res


def kernel(x_i, x_j):
    loss, _ = run(x_i, x_j, trace=False)
    return loss
